# revision 9
# baseline (speedup 1.0000x reference)
"""BitLinear forward on 8 TRN2 NeuronCores (tensor-parallel, column-parallel linear).

  alpha = mean(|W|)            (scalar over the FULL weight matrix)
  y     = x @ (sign(W) * alpha)^T

Sharding: W rows (out_features) split across 8 cores; x replicated; each core
computes y[:, c*2048:(c+1)*2048]. alpha is a scalar reduction over the local
shard on each core, combined across shards between the two launches (summing 8
partial scalars; the device does all O(n) work).

Two SPMD launches (a real 8-rank collective_compute in the NEFF permanently
downclocks the PE from 2.4GHz to ~2.0GHz for the whole run, costing ~22% on
every matmul — so the cross-core scalar reduction is NOT done with a
collective):

  Kernel A (prep): per core, load W shard fp32, sign() -> bf16, PE-transpose
    into K-major layout; k-blocks 0..KF-1 stored as fp8e4 (+-1 exact), blocks
    KF..31 as bf16; also |W| row-sums -> partition_all_reduce -> scalar
    partial sum output.
  Kernel B (main): load wt8/wtb into SBUF once; broadcast alpha; per 128-row
    x tile: load fp32 -> cast bf16 -> SBUF->SBUF XBAR DMA-transpose ->
    xT [128, 32, 128]; cast blocks 0..KF-1 to fp8 -> xT8; per psum j-chunk:
    KF/2 fp8 DoubleRow pair-matmuls (256-row contraction each) + (32-KF) bf16
    matmuls accumulate [128, 2048] fp32 in PSUM; ScalarE Copy*alpha eviction;
    DMA out.

Precision: x quantized to fp8e4m3 on KF=16 of 32 k-blocks (measured DoubleRow
per-matmul cost == bf16 cost at N=512 => 2x throughput on those blocks).
Weights are sign() -> +-1, exact in both fp8 and bf16. Simulated end-to-end
rel l2 err 1.89e-2 (gate 2e-2); the bf16-only path measured 1.66e-3.

Known pitfalls (verified on HW): XBAR transposes must all issue from nc.sync
(issuing some from nc.scalar corrupts data); removing "redundant" per-matmul
LDWEIGHTS corrupts results (PE weight-buffer management assumes self-loading);
a real multi-rank collective_compute downclocks the PE for the entire NEFF.
"""
import sys
import os

sys.path.insert(0, "/opt/trn_rl_repo")
import numpy as np

P = 128
S, I, O = 8192, 4096, 16384
N_CORES = 8
OC = O // N_CORES          # 2048 out-features per core
KB = I // P                # 32 contraction blocks
KF = 16                    # k-blocks carried in fp8 (DoubleRow pairs)
NT = S // P                # 64 x row-tiles
NJ = OC // 512             # 4 psum bank chunks

_cache = {}


def _build_prep():
    from concourse import bacc, tile, mybir, bass_isa
    from concourse.masks import make_identity

    dt = mybir.dt
    nc = bacc.Bacc("TRN2", target_bir_lowering=False, debug=False, num_devices=N_CORES)
    w_ap = nc.dram_tensor("w", [OC, I], dt.float32, kind="ExternalInput").ap()
    w8_ap = nc.dram_tensor("wt8", [P, KF, OC], dt.float8e4, kind="ExternalOutput").ap()
    wb_ap = nc.dram_tensor("wtb", [P, KB - KF, OC], dt.bfloat16, kind="ExternalOutput").ap()
    as_ap = nc.dram_tensor("asum", [1, 1], dt.float32, kind="ExternalOutput").ap()

    HI = I // 2
    HB = KB // 2
    assert KF == HB, "prep assumes the fp8 half is exactly k-blocks 0..15"

    with tile.TileContext(nc) as tc:
        with (
            tc.tile_pool(name="pers", bufs=1) as pers,
            tc.tile_pool(name="wld", bufs=8) as wld,
            tc.tile_pool(name="wsg", bufs=4) as wsg,
            tc.tile_pool(name="psum", bufs=4, space="PSUM") as psum,
        ):
            ident = pers.tile([P, P], dt.bfloat16)
            make_identity(nc, ident)
            WT8 = pers.tile([P, KF, OC], dt.float8e4)
            WTB = pers.tile([P, KB - KF, OC], dt.bfloat16)
            wabs = pers.tile([P, 2 * (OC // P)], dt.float32)
            for h in range(2):
                for t in range(OC // P):
                    w32 = wld.tile([P, HI], dt.float32, tag="wld")
                    nc.sync.dma_start(w32[:], w_ap[t * P:(t + 1) * P, h * HI:(h + 1) * HI])
                    sg = wsg.tile([P, HI], dt.bfloat16, tag="wsg")
                    nc.scalar.sign(sg[:], w32[:])
                    nc.vector.tensor_reduce(
                        wabs[:, 2 * t + h:2 * t + h + 1], w32[:],
                        axis=mybir.AxisListType.XYZW,
                        op=mybir.AluOpType.add, apply_absolute_value=True)
                    psT = psum.tile([P, HB, P], dt.bfloat16, tag="ps")
                    for b in range(HB):
                        nc.tensor.transpose(psT[:, b, :], sg[:, b * P:(b + 1) * P], ident[:])
                    if h == 0:
                        wt_dst = WT8[:, :, t * P:(t + 1) * P]
                    else:
                        wt_dst = WTB[:, :, t * P:(t + 1) * P]
                    if t % 2 == 0:
                        nc.scalar.activation(wt_dst, psT[:],
                                             mybir.ActivationFunctionType.Copy)
                    else:
                        nc.vector.tensor_copy(wt_dst, psT[:])
                    # piecewise stores on the scalar queue: keeps store traffic
                    # off the load queue, and halves the exposed tail of the
                    # final wtb store
                    if h == 0 and t == OC // P - 1:
                        nc.scalar.dma_start(w8_ap, WT8[:])
                    elif h == 1 and t == OC // P // 2 - 1:
                        nc.scalar.dma_start(wb_ap[:, :, 0:OC // 2], WTB[:, :, 0:OC // 2])
                    elif h == 1 and t == OC // P - 1:
                        nc.scalar.dma_start(wb_ap[:, :, OC // 2:], WTB[:, :, OC // 2:])
            wsum = pers.tile([P, 1], dt.float32)
            nc.vector.tensor_reduce(
                wsum[:], wabs[:], axis=mybir.AxisListType.XYZW,
                op=mybir.AluOpType.add)
            par = pers.tile([P, 1], dt.float32)
            nc.gpsimd.partition_all_reduce(
                par[:], wsum[:], channels=P, reduce_op=bass_isa.ReduceOp.add)
            nc.sync.dma_start(as_ap, par[0:1, :])

    nc.compile()
    return nc


def _build_main():
    from concourse import bacc, tile, mybir

    dt = mybir.dt
    DR = mybir.MatmulPerfMode.DoubleRow
    nc = bacc.Bacc("TRN2", target_bir_lowering=False, debug=False, num_devices=N_CORES)
    x_ap = nc.dram_tensor("x", [S, I], dt.float32, kind="ExternalInput").ap()
    w8_ap = nc.dram_tensor("wt8", [P, KF, OC], dt.float8e4, kind="ExternalInput").ap()
    wb_ap = nc.dram_tensor("wtb", [P, KB - KF, OC], dt.bfloat16, kind="ExternalInput").ap()
    al_ap = nc.dram_tensor("al", [P, 1], dt.float32, kind="ExternalInput").ap()
    y_ap = nc.dram_tensor("y", [S, OC], dt.float32, kind="ExternalOutput").ap()

    with tile.TileContext(nc) as tc:
        with (
            tc.tile_pool(name="pers", bufs=1) as pers,
            tc.tile_pool(name="xld", bufs=2) as xld,
            tc.tile_pool(name="xsg", bufs=2) as xsg,
            tc.tile_pool(name="pxT", bufs=4) as pxT,
            tc.tile_pool(name="px8", bufs=4) as px8,
            tc.tile_pool(name="pyo", bufs=2) as pyo,
            tc.tile_pool(name="psum", bufs=2, space="PSUM") as psum,
        ):
            # alpha first: the x casts fold alpha*2^7 in, so it must be ready
            # before the first tile's cast. The host passes it pre-scaled and
            # pre-broadcast to [P, 1] (one tiny DMA, no gpsimd dependency).
            alpha = pers.tile([P, 1], dt.float32)
            nc.sync.dma_start(alpha[:], al_ap)
            # fully prepare the first x tiles (incl. their XBAR transposes)
            # BEFORE the bulk WT load: a DMA-transpose serializes against all
            # in-flight plain DMAs (xbar mode switch), so issuing xT0 after the
            # 12MB WT load would stall it ~20us
            NPRE = 3
            preT = []
            for st in range(NPRE):
                x32 = xld.tile([P, I], dt.float32, tag="xld")
                nc.sync.dma_start(x32[:], x_ap[st * P:(st + 1) * P, :])
                xc = xsg.tile([P, I], dt.bfloat16, tag="xsg")
                nc.vector.tensor_scalar_mul(xc[:], x32[:], alpha[:, 0:1])
                xT = pxT.tile([P, KB, P], dt.bfloat16, tag="xT")
                nc.sync.dma_start_transpose(xT[:], xc[:])
                x8 = px8.tile([P, KF, P], dt.float8e4, tag="x8")
                nc.vector.tensor_copy(x8[:], xT[:, :KF, :])
                preT.append((xT, x8))
            # WT loads issue on the sync queue AFTER the prefetch transposes:
            # an XBAR transpose serializes against ALL in-flight plain DMAs
            # (global xbar mode switch), so WT traffic on any ring before the
            # early transposes would stall them ~35us
            WT8 = pers.tile([P, KF, OC], dt.float8e4)
            for c in range(4):
                # chunked so the first matmuls only wait for the first piece
                nc.sync.dma_start(WT8[:, 4 * c:4 * (c + 1), :], w8_ap[:, 4 * c:4 * (c + 1), :])
            WTB = pers.tile([P, KB - KF, OC], dt.bfloat16)
            for c in range(4):
                nc.sync.dma_start(WTB[:, 4 * c:4 * (c + 1), :], wb_ap[:, 4 * c:4 * (c + 1), :])

            for st in range(NT):
                if st < NPRE:
                    xT, x8 = preT[st]
                else:
                    x32 = xld.tile([P, I], dt.float32, tag="xld")
                    nc.sync.dma_start(x32[:], x_ap[st * P:(st + 1) * P, :])
                    xc = xsg.tile([P, I], dt.bfloat16, tag="xsg")
                    nc.vector.tensor_scalar_mul(xc[:], x32[:], alpha[:, 0:1])
                    xT = pxT.tile([P, KB, P], dt.bfloat16, tag="xT")
                    nc.sync.dma_start_transpose(xT[:], xc[:])
                    x8 = px8.tile([P, KF, P], dt.float8e4, tag="x8")
                    nc.vector.tensor_copy(x8[:], xT[:, :KF, :])
                ps = psum.tile([P, OC], dt.float32, tag="ps")
                for g in range(KF // 2):
                    for j in range(NJ):
                        nc.tensor.matmul(
                            ps[:, j * 512:(j + 1) * 512],
                            x8[:, 2 * g:2 * g + 2, :],
                            WT8[:, 2 * g:2 * g + 2, j * 512:(j + 1) * 512],
                            start=(g == 0), stop=False, perf_mode=DR)
                for k in range(KF, KB):
                    for j in range(NJ):
                        nc.tensor.matmul(
                            ps[:, j * 512:(j + 1) * 512],
                            xT[:, k, :],
                            WTB[:, k - KF, j * 512:(j + 1) * 512],
                            start=False, stop=(k == KB - 1))
                yo = pyo.tile([P, OC], dt.float32, tag="yo")
                # x carried alpha*2^7; undo the exact power-of-two lift with an
                # immediate scale (the vector-scale activation path is ~10x
                # slower and was nearly co-critical with the PE)
                nc.scalar.activation(
                    yo[:], ps[:], mybir.ActivationFunctionType.Copy,
                    bias=0.0, scale=1.0 / 128.0)
                nc.scalar.dma_start(y_ap[st * P:(st + 1) * P, :], yo[:])

    nc.compile()
    return nc


def _get_ncs():
    if "nc_main" not in _cache:
        _cache["nc_prep"] = _build_prep()
        _cache["nc_main"] = _build_main()
    return _cache["nc_prep"], _cache["nc_main"]


def kernel(x: np.ndarray, weight: np.ndarray) -> np.ndarray:
    from concourse.bass_utils import run_bass_kernel_spmd

    nc_prep, nc_main = _get_ncs()
    trace = bool(int(os.environ.get("BITLINEAR_TRACE", "0")))

    wf = np.asarray(weight, dtype=np.float32)
    in_a = [{"w": np.ascontiguousarray(wf[c * OC:(c + 1) * OC])} for c in range(N_CORES)]
    res_a = run_bass_kernel_spmd(nc_prep, in_a, core_ids=list(range(N_CORES)), trace=trace)

    total = np.float32(sum(res_a.results[c]["asum"][0, 0] for c in range(N_CORES)))
    # alpha * 2^7: folded into the x cast on device; evictions undo the exact
    # power-of-two lift with an immediate 1/128 scale
    alpha_t = np.float32(total) * np.float32(128.0 / (float(O) * float(I)))
    al = np.full((P, 1), alpha_t, dtype=np.float32)

    xf = np.ascontiguousarray(np.asarray(x, dtype=np.float32).reshape(S, I))
    in_b = [
        {"x": xf, "wt8": res_a.results[c]["wt8"], "wtb": res_a.results[c]["wtb"], "al": al}
        for c in range(N_CORES)
    ]
    res_b = run_bass_kernel_spmd(nc_main, in_b, core_ids=list(range(N_CORES)), trace=trace)

    _cache["exec_time_ns_prep"] = res_a.exec_time_ns
    _cache["exec_time_ns_main"] = res_b.exec_time_ns
    if res_a.exec_time_ns is not None and res_b.exec_time_ns is not None:
        _cache["exec_time_ns"] = res_a.exec_time_ns + res_b.exec_time_ns
    y = np.concatenate([res_b.results[c]["y"] for c in range(N_CORES)], axis=1)
    return y.reshape(2, S // 2, O)


# revision 11
# speedup vs baseline: 1.1804x; 1.1804x over previous
"""BitLinear forward on 8 TRN2 NeuronCores (tensor-parallel, column-parallel linear).

  alpha = mean(|W|)            (scalar over the FULL weight matrix)
  y     = x @ (sign(W) * alpha)^T

Sharding: W rows (out_features) split across 8 cores; x replicated; each core
computes y[:, c*2048:(c+1)*2048]. alpha is a scalar reduction over the local
shard on each core, combined across shards between the two launches (summing 8
partial scalars; the device does all O(n) work).

Two SPMD launches (a real 8-rank collective_compute in the NEFF permanently
downclocks the PE from 2.4GHz to ~2.0GHz for the whole run, costing ~22% on
every matmul — so the cross-core scalar reduction is NOT done with a
collective):

  Kernel A (prep, ~140us): per core, load W shard fp32, sign() -> bf16,
    PE-transpose into K-major layout; k-blocks 0..KF-1 stored as fp8e4
    (+-1 exact), blocks KF..31 as bf16; |W| row-sums (DVE) ->
    partition_all_reduce -> scalar partial sum output. Stores issue on the
    scalar ring (off the load ring), wtb in two pieces to hide the tail.
  Kernel B (main, ~1.40ms): host passes alpha*2^7 pre-broadcast [128,1];
    per 128-row x tile: load fp32 -> DVE tensor_scalar_mul by alpha*2^7
    (fold the scalar in while casting to bf16) -> SBUF->SBUF XBAR
    DMA-transpose -> xT [128, 32, 128]; DVE-cast blocks 0..KF-1 to fp8 ->
    x8; per psum j-chunk: KF/2 fp8e4 DoubleRow pair-matmuls (256-row
    contraction each) + (32-KF) bf16 matmuls accumulate [128, 2048] fp32 in
    PSUM; ScalarE Copy eviction with IMMEDIATE scale 1/128 (exact
    power-of-two undo); DMA out.

Why this is fast (all HW-measured on trn2):
  - fp8e4xfp8e4 perf_mode=DoubleRow costs the same 216ns per N=512 matmul as
    bf16 (2x contraction per slot), so the 16 fp8 k-blocks take 8 slots.
    96 matmul slots/tile instead of 128 -> PE floor 1.33ms vs 1.77ms.
  - activation() with a per-partition VECTOR scale runs ~10x slower than with
    an immediate scale (20.7us vs 2us per [128,2048] eviction); folding alpha
    into the x cast (free on DVE) and evicting with immediate 1/128 keeps
    ScalarE off the critical path.
  - alpha arrives pre-broadcast from the host: no gpsimd partition_broadcast
    blocking the WT dma ring at startup.

Precision: x quantized to fp8e4m3 on KF=16 of 32 k-blocks. Weights are
sign() -> +-1, exact in both fp8 and bf16; products fp8*{+-1} are exact, so
the only error is x-quantization. Measured end-to-end rel l2 err 1.888e-2
(gate 2e-2), bit-matching the numpy simulation; KF=14 (1.77e-2) is the
fallback margin knob. KF=18 fails (2.0015e-2).

Known pitfalls (verified on HW): XBAR transposes must all issue from nc.sync
(issuing some from nc.scalar corrupts data); an XBAR transpose serializes
against ALL in-flight plain DMAs on every ring (global xbar mode switch);
removing "redundant" per-matmul LDWEIGHTS corrupts results (PE weight-buffer
management assumes self-loading); a real multi-rank collective_compute
downclocks the PE from 2.4 to ~2.0GHz for the entire NEFF (so the cross-core
alpha reduction goes through the host between launches); GPSIMD has no PSUM
port; gpsimd tensor_reduce only does partition-axis reductions; DMA cannot
touch PSUM; dma_start_transpose requires a 2-byte dtype; sustained runs can
enter power state P0 (PE ~2.0GHz), adding ~20% run-to-run variance.
"""
import sys
import os

sys.path.insert(0, "/opt/trn_rl_repo")
import numpy as np

P = 128
S, I, O = 8192, 4096, 16384
N_CORES = 8
OC = O // N_CORES          # 2048 out-features per core
KB = I // P                # 32 contraction blocks
KF = 16                    # k-blocks carried in fp8 (DoubleRow pairs)
NT = S // P                # 64 x row-tiles
NJ = OC // 512             # 4 psum bank chunks

_cache = {}


def _build_prep():
    from concourse import bacc, tile, mybir, bass_isa
    from concourse.masks import make_identity

    dt = mybir.dt
    nc = bacc.Bacc("TRN2", target_bir_lowering=False, debug=False, num_devices=N_CORES)
    w_ap = nc.dram_tensor("w", [OC, I], dt.float32, kind="ExternalInput").ap()
    w8_ap = nc.dram_tensor("wt8", [P, KF, OC], dt.float8e4, kind="ExternalOutput").ap()
    wb_ap = nc.dram_tensor("wtb", [P, KB - KF, OC], dt.bfloat16, kind="ExternalOutput").ap()
    as_ap = nc.dram_tensor("asum", [1, 1], dt.float32, kind="ExternalOutput").ap()

    HI = I // 2
    HB = KB // 2
    assert KF == HB, "prep assumes the fp8 half is exactly k-blocks 0..15"

    with tile.TileContext(nc) as tc:
        with (
            tc.tile_pool(name="pers", bufs=1) as pers,
            tc.tile_pool(name="wld", bufs=8) as wld,
            tc.tile_pool(name="wsg", bufs=4) as wsg,
            tc.tile_pool(name="psum", bufs=4, space="PSUM") as psum,
        ):
            ident = pers.tile([P, P], dt.bfloat16)
            make_identity(nc, ident)
            WT8 = pers.tile([P, KF, OC], dt.float8e4)
            WTB = pers.tile([P, KB - KF, OC], dt.bfloat16)
            wabs = pers.tile([P, 2 * (OC // P)], dt.float32)
            for h in range(2):
                for t in range(OC // P):
                    w32 = wld.tile([P, HI], dt.float32, tag="wld")
                    nc.sync.dma_start(w32[:], w_ap[t * P:(t + 1) * P, h * HI:(h + 1) * HI])
                    sg = wsg.tile([P, HI], dt.bfloat16, tag="wsg")
                    nc.scalar.sign(sg[:], w32[:])
                    nc.vector.tensor_reduce(
                        wabs[:, 2 * t + h:2 * t + h + 1], w32[:],
                        axis=mybir.AxisListType.XYZW,
                        op=mybir.AluOpType.add, apply_absolute_value=True)
                    psT = psum.tile([P, HB, P], dt.bfloat16, tag="ps")
                    for b in range(HB):
                        nc.tensor.transpose(psT[:, b, :], sg[:, b * P:(b + 1) * P], ident[:])
                    if h == 0:
                        wt_dst = WT8[:, :, t * P:(t + 1) * P]
                    else:
                        wt_dst = WTB[:, :, t * P:(t + 1) * P]
                    if t % 2 == 0:
                        nc.scalar.activation(wt_dst, psT[:],
                                             mybir.ActivationFunctionType.Copy)
                    else:
                        nc.vector.tensor_copy(wt_dst, psT[:])
                    # piecewise stores on the scalar queue: keeps store traffic
                    # off the load queue, and halves the exposed tail of the
                    # final wtb store
                    if h == 0 and t == OC // P - 1:
                        nc.scalar.dma_start(w8_ap, WT8[:])
                    elif h == 1 and t == OC // P // 2 - 1:
                        nc.scalar.dma_start(wb_ap[:, :, 0:OC // 2], WTB[:, :, 0:OC // 2])
                    elif h == 1 and t == OC // P - 1:
                        nc.scalar.dma_start(wb_ap[:, :, OC // 2:], WTB[:, :, OC // 2:])
            wsum = pers.tile([P, 1], dt.float32)
            nc.vector.tensor_reduce(
                wsum[:], wabs[:], axis=mybir.AxisListType.XYZW,
                op=mybir.AluOpType.add)
            par = pers.tile([P, 1], dt.float32)
            nc.gpsimd.partition_all_reduce(
                par[:], wsum[:], channels=P, reduce_op=bass_isa.ReduceOp.add)
            nc.sync.dma_start(as_ap, par[0:1, :])

    nc.compile()
    return nc


def _build_main():
    from concourse import bacc, tile, mybir

    dt = mybir.dt
    DR = mybir.MatmulPerfMode.DoubleRow
    nc = bacc.Bacc("TRN2", target_bir_lowering=False, debug=False, num_devices=N_CORES)
    x_ap = nc.dram_tensor("x", [S, I], dt.float32, kind="ExternalInput").ap()
    w8_ap = nc.dram_tensor("wt8", [P, KF, OC], dt.float8e4, kind="ExternalInput").ap()
    wb_ap = nc.dram_tensor("wtb", [P, KB - KF, OC], dt.bfloat16, kind="ExternalInput").ap()
    al_ap = nc.dram_tensor("al", [P, 1], dt.float32, kind="ExternalInput").ap()
    y_ap = nc.dram_tensor("y", [S, OC], dt.float32, kind="ExternalOutput").ap()

    with tile.TileContext(nc) as tc:
        with (
            tc.tile_pool(name="pers", bufs=1) as pers,
            tc.tile_pool(name="xld", bufs=2) as xld,
            tc.tile_pool(name="xsg", bufs=2) as xsg,
            tc.tile_pool(name="pxT", bufs=4) as pxT,
            tc.tile_pool(name="px8", bufs=4) as px8,
            tc.tile_pool(name="pyo", bufs=2) as pyo,
            tc.tile_pool(name="psum", bufs=2, space="PSUM") as psum,
        ):
            # alpha first: the x casts fold alpha*2^7 in, so it must be ready
            # before the first tile's cast. The host passes it pre-scaled and
            # pre-broadcast to [P, 1] (one tiny DMA, no gpsimd dependency).
            alpha = pers.tile([P, 1], dt.float32)
            nc.sync.dma_start(alpha[:], al_ap)
            # fully prepare the first x tiles (incl. their XBAR transposes)
            # BEFORE the bulk WT load: a DMA-transpose serializes against all
            # in-flight plain DMAs (xbar mode switch), so issuing xT0 after the
            # 12MB WT load would stall it ~20us
            NPRE = 3
            preT = []
            for st in range(NPRE):
                x32 = xld.tile([P, I], dt.float32, tag="xld")
                nc.sync.dma_start(x32[:], x_ap[st * P:(st + 1) * P, :])
                xc = xsg.tile([P, I], dt.bfloat16, tag="xsg")
                nc.vector.tensor_scalar_mul(xc[:], x32[:], alpha[:, 0:1])
                xT = pxT.tile([P, KB, P], dt.bfloat16, tag="xT")
                nc.sync.dma_start_transpose(xT[:], xc[:])
                x8 = px8.tile([P, KF, P], dt.float8e4, tag="x8")
                nc.vector.tensor_copy(x8[:], xT[:, :KF, :])
                preT.append((xT, x8))
            # WT loads go on the gpsimd DMA ring, concurrent with x loads on
            # the sync ring. The early XBAR transposes still pay the global
            # xbar-vs-plain-DMA serialization against the in-flight WT bulk
            # (~40us lead-in before the first matmul); orderings that avoid it
            # were measured no better because tile 0's bf16 matmuls need all
            # 8MB of WTB within ~40us anyway.
            WT8 = pers.tile([P, KF, OC], dt.float8e4)
            for c in range(4):
                # chunked so the first matmuls only wait for the first piece
                nc.gpsimd.dma_start(WT8[:, 4 * c:4 * (c + 1), :], w8_ap[:, 4 * c:4 * (c + 1), :])
            WTB = pers.tile([P, KB - KF, OC], dt.bfloat16)
            for c in range(4):
                nc.gpsimd.dma_start(WTB[:, 4 * c:4 * (c + 1), :], wb_ap[:, 4 * c:4 * (c + 1), :])

            for st in range(NT):
                if st < NPRE:
                    xT, x8 = preT[st]
                else:
                    x32 = xld.tile([P, I], dt.float32, tag="xld")
                    nc.sync.dma_start(x32[:], x_ap[st * P:(st + 1) * P, :])
                    xc = xsg.tile([P, I], dt.bfloat16, tag="xsg")
                    nc.vector.tensor_scalar_mul(xc[:], x32[:], alpha[:, 0:1])
                    xT = pxT.tile([P, KB, P], dt.bfloat16, tag="xT")
                    nc.sync.dma_start_transpose(xT[:], xc[:])
                    x8 = px8.tile([P, KF, P], dt.float8e4, tag="x8")
                    nc.vector.tensor_copy(x8[:], xT[:, :KF, :])
                ps = psum.tile([P, OC], dt.float32, tag="ps")
                for g in range(KF // 2):
                    for j in range(NJ):
                        nc.tensor.matmul(
                            ps[:, j * 512:(j + 1) * 512],
                            x8[:, 2 * g:2 * g + 2, :],
                            WT8[:, 2 * g:2 * g + 2, j * 512:(j + 1) * 512],
                            start=(g == 0), stop=False, perf_mode=DR)
                for k in range(KF, KB):
                    for j in range(NJ):
                        nc.tensor.matmul(
                            ps[:, j * 512:(j + 1) * 512],
                            xT[:, k, :],
                            WTB[:, k - KF, j * 512:(j + 1) * 512],
                            start=False, stop=(k == KB - 1))
                yo = pyo.tile([P, OC], dt.float32, tag="yo")
                # x carried alpha*2^7; undo the exact power-of-two lift with an
                # immediate scale (the vector-scale activation path is ~10x
                # slower and was nearly co-critical with the PE)
                nc.scalar.activation(
                    yo[:], ps[:], mybir.ActivationFunctionType.Copy,
                    bias=0.0, scale=1.0 / 128.0)
                nc.scalar.dma_start(y_ap[st * P:(st + 1) * P, :], yo[:])

    nc.compile()
    return nc


def _get_ncs():
    if "nc_main" not in _cache:
        _cache["nc_prep"] = _build_prep()
        _cache["nc_main"] = _build_main()
    return _cache["nc_prep"], _cache["nc_main"]


def kernel(x: np.ndarray, weight: np.ndarray) -> np.ndarray:
    from concourse.bass_utils import run_bass_kernel_spmd

    nc_prep, nc_main = _get_ncs()
    trace = bool(int(os.environ.get("BITLINEAR_TRACE", "0")))

    wf = np.asarray(weight, dtype=np.float32)
    in_a = [{"w": np.ascontiguousarray(wf[c * OC:(c + 1) * OC])} for c in range(N_CORES)]
    res_a = run_bass_kernel_spmd(nc_prep, in_a, core_ids=list(range(N_CORES)), trace=trace)

    total = np.float32(sum(res_a.results[c]["asum"][0, 0] for c in range(N_CORES)))
    # alpha * 2^7: folded into the x cast on device; evictions undo the exact
    # power-of-two lift with an immediate 1/128 scale
    alpha_t = np.float32(total) * np.float32(128.0 / (float(O) * float(I)))
    al = np.full((P, 1), alpha_t, dtype=np.float32)

    xf = np.ascontiguousarray(np.asarray(x, dtype=np.float32).reshape(S, I))
    in_b = [
        {"x": xf, "wt8": res_a.results[c]["wt8"], "wtb": res_a.results[c]["wtb"], "al": al}
        for c in range(N_CORES)
    ]
    res_b = run_bass_kernel_spmd(nc_main, in_b, core_ids=list(range(N_CORES)), trace=trace)

    _cache["exec_time_ns_prep"] = res_a.exec_time_ns
    _cache["exec_time_ns_main"] = res_b.exec_time_ns
    if res_a.exec_time_ns is not None and res_b.exec_time_ns is not None:
        _cache["exec_time_ns"] = res_a.exec_time_ns + res_b.exec_time_ns
    y = np.concatenate([res_b.results[c]["y"] for c in range(N_CORES)], axis=1)
    return y.reshape(2, S // 2, O)
